# revision 15
# baseline (speedup 1.0000x reference)
"""ARMAPlusConv (10-step symmetric-normalized diffusion + fused linear head)
as a Trainium2 Bass/Tile kernel running SPMD on 8 NeuronCores.

Self-contained: kernel(**inputs) takes the full unsharded inputs
(x [N,128] f32, edge_index [2,E] i64, t [10,128] f32, init_weight [1,128,128],
root_weight [1,1,128,128], bias [1,1,1,128]) and returns the full
[N,128] f32 output.

Math (K=1; mean over K is the identity):
  deg[d] = 1 + indeg(d);  dinv = deg**-0.5
  v_0 = dinv * x
  v_s[d] = dinv2[d] * (sum_{src->d} v_{s-1}[src] + v_{s-1}[d]),  s=1..9
  y = (1/dinv) * sum_s tnorm[s] * v_s     (tnorm = softmax(t, axis=0))
  out = relu(y @ (Wi + Wr) + bias)

Distribution: nodes are dest-sharded over the 8 cores (S rows each, padded
with fake zero rows). v is carried in fp16 (tolerance 2e-2 allows it): this
halves the gather HBM traffic, the AllGather bytes, and doubles the DVE add
throughput. Each step, every core gathers its in-edge source rows from a
full replicated fp16 copy of v in its DRAM (refreshed by an AllGather of
the updated shards), segment-sums them with DVE adds, applies the self loop
and scales, and accumulates the softmax-weighted result in f32.

The sparse gather uses GPSIMD dma_gather (int16 indices, CHUNK per call,
4 SWDGE queues). Sources are split into two windows (cores 0-3 / cores 4-7)
so row indices fit int16. Each window uses degree-sorted prefix slots: slot
k holds the k-th in-edge source of every dest that has one; dests are
sorted by window-degree so slot k's dest list is a prefix and the gather
output (stream position q -> partition q%128, block q//128) lines up with
the accumulator exactly. Window A's dest order is the storage order; window
B accumulates separately (accB) in its own order and is merged back with
one S-row permute gather per step.

All bulk DRAM buffers (x, v shards, accB, out) use the node-major on-chip
layout [P, NB, 128] flattened to rows "(p b) f" so every bulk DMA is fully
contiguous; the gather index tables are built against that row numbering.
"""
import sys
sys.path.insert(0, "/opt/trn_rl_repo")
import numpy as np

P = 128
NCORES = 8
STEP = 10
CHUNK = 1024
NQ = 4
SCRATCH = 16384


# ---------------------------------------------------------------------------
# host preprocessing (graph structure only)
# ---------------------------------------------------------------------------

def _preprocess(edge_index, N):
    S = int(np.ceil(N / (NCORES * P))) * P
    Npad = NCORES * S
    HALF = (NCORES // 2) * S
    NB = S // P

    row = np.asarray(edge_index[0], dtype=np.int64)
    col = np.asarray(edge_index[1], dtype=np.int64)

    indeg = np.bincount(col, minlength=N)
    dinv_real = ((indeg + 1.0) ** -0.5).astype(np.float32)

    order = np.argsort(-indeg, kind="stable")
    rank_node = np.concatenate([order, np.arange(N, Npad)])
    members = [rank_node[np.arange(Npad) % NCORES == c] for c in range(NCORES)]
    node2core = np.empty(Npad, dtype=np.int64)
    for c in range(NCORES):
        node2core[members[c]] = c

    isA = node2core[row] < (NCORES // 2)
    degA = np.bincount(col[isA], minlength=Npad)
    degB = np.bincount(col[~isA], minlength=Npad)
    is_fake = np.zeros(Npad, dtype=bool)
    is_fake[N:] = True

    # storage slot s (0..S-1) of a node within its core; A-order == storage.
    stor_of_node = np.empty(Npad, dtype=np.int64)
    node_of_stor = np.empty(Npad, dtype=np.int64)
    piB_of_node = np.empty(Npad, dtype=np.int64)
    for c in range(NCORES):
        m = members[c]
        mA = m[np.lexsort((m, is_fake[m], -degA[m]))]
        stor_of_node[mA] = c * S + np.arange(S)
        node_of_stor[c * S + np.arange(S)] = mA
        mB = m[np.lexsort((m, is_fake[m], -degB[m]))]
        piB_of_node[mB] = np.arange(S)

    # DRAM row index of a storage slot: shard layout [P, NB, 128] flattened,
    # slot s -> partition s%P, block s//P -> row (s%P)*NB + s//P.
    def dram_row(slot):
        return (slot % P) * NB + slot // P

    st_src = stor_of_node[row]
    st_dst = stor_of_node[col]

    def build_window(mask, local_pos, deg_w, idx_base, zpad_stor):
        d_sel = st_dst[mask]
        s_sel = st_src[mask]
        o = np.argsort(d_sel, kind="stable")
        e_dst = d_sel[o]
        e_src = s_sel[o]
        starts = np.searchsorted(e_dst, np.arange(Npad))
        kk = np.arange(len(e_dst)) - starts[e_dst]
        Kmax = int(kk.max()) + 1 if len(kk) else 0
        pref = np.zeros((NCORES, Kmax), dtype=np.int64)
        for c in range(NCORES):
            dw = deg_w[members[c]]
            cnt = np.bincount(np.minimum(dw, Kmax), minlength=Kmax + 1)
            tail = np.cumsum(cnt[::-1])[::-1]
            pref[c] = tail[1:Kmax + 1]
        PK = ((pref.max(axis=0) + P - 1) // P) * P
        off = np.zeros(Kmax + 1, dtype=np.int64)
        off[1:] = np.cumsum(PK)
        # gather row index within the window: shard-local node-major rows.
        def win_row(stor):
            sl = stor - idx_base  # position within the window (0..HALF)
            core_off = (sl // S) * S
            return core_off + dram_row(sl % S)
        fill = win_row(np.int64(zpad_stor))
        tabs = np.full((NCORES, int(off[-1])), fill, dtype=np.int64)
        tabs[e_dst // S, off[kk] + local_pos[node_of_stor[e_dst]]] = \
            win_row(e_src)
        assert tabs.min() >= 0 and tabs.max() < HALF
        return tabs.astype(np.int16), PK

    posA = np.empty(Npad, dtype=np.int64)
    for c in range(NCORES):
        posA[node_of_stor[c * S:(c + 1) * S]] = np.arange(S)

    tabsA, PKA = build_window(isA, posA, degA, 0, HALF - 1)
    tabsB, PKB = build_window(~isA, piB_of_node, degB, HALF, Npad - 1)
    assert is_fake[node_of_stor[HALF - 1]] and is_fake[node_of_stor[Npad - 1]]

    # merge: storage slot s of core c holds node n with B-position piB(n);
    # accB_dram rows are node-major in B-position.
    merge_idx = np.empty((NCORES, S), dtype=np.int16)
    for c in range(NCORES):
        merge_idx[c] = dram_row(
            piB_of_node[node_of_stor[c * S:(c + 1) * S]]).astype(np.int16)

    nid = node_of_stor
    real = nid < N
    dinv_st = np.zeros(Npad, dtype=np.float32)
    dinv_st[real] = dinv_real[nid[real]]
    dinvinv_st = np.zeros(Npad, dtype=np.float32)
    dinvinv_st[real] = 1.0 / dinv_real[nid[real]]

    def node_major(a, c):
        return a[c * S:(c + 1) * S].reshape(NB, P).T.copy()

    return dict(
        N=N, S=S, Npad=Npad, HALF=HALF, NB=NB,
        node_of_stor=node_of_stor,
        tabsA=tabsA, PKA=PKA, tabsB=tabsB, PKB=PKB, merge_idx=merge_idx,
        dinv_nm=[node_major(dinv_st, c) for c in range(NCORES)],
        dinv2_nm=[node_major(dinv_st * dinv_st, c) for c in range(NCORES)],
        dinvinv_nm=[node_major(dinvinv_st, c) for c in range(NCORES)],
    )


def _chunk_plan(PK):
    offs = np.zeros(len(PK) + 1, dtype=np.int64)
    offs[1:] = np.cumsum(PK)
    TOT = int(offs[-1])
    plan = []
    t0 = 0
    while t0 < TOT:
        t1 = min(t0 + CHUNK, TOT)
        pieces = []
        for k, pk in enumerate(PK):
            a = max(int(offs[k]), t0)
            b = min(int(offs[k]) + int(pk), t1)
            if a < b:
                pieces.append((k, a - t0, a - int(offs[k]), b - a))
        plan.append((t0, t1, pieces))
        t0 = t1
    return plan, TOT


def _wrap_idx(flat):
    L = len(flat)
    return np.tile(flat.reshape(L // 16, 16).T, (8, 1)).copy()


# ---------------------------------------------------------------------------
# device program
# ---------------------------------------------------------------------------

def _build_program(meta, reps=1, mock_collectives=False,
                   skip_adds=False, skip_gathers=False):
    from concourse import mybir, tile, bacc

    S = meta["S"]; HALF = meta["HALF"]; Npad = meta["Npad"]
    NB = S // P
    f32 = mybir.dt.float32
    f16 = mybir.dt.float16

    planA, TOTA = _chunk_plan(meta["PKA"])
    planB, TOTB = _chunk_plan(meta["PKB"])
    TOT = TOTA + TOTB + S

    nc = bacc.Bacc("TRN2", target_bir_lowering=False, debug=False,
                   num_devices=NCORES, num_swdge_queues=NQ,
                   dynamic_dma_scratch_size=SCRATCH)

    x_in = nc.dram_tensor("x", [S, P], f16, kind="ExternalInput")
    idx_in = nc.dram_tensor("idx", [128, TOT // 16], mybir.dt.int16,
                            kind="ExternalInput")
    sc_in = nc.dram_tensor("scales", [P, 3, NB], f32, kind="ExternalInput")
    tn_in = nc.dram_tensor("tnorm", [P, STEP, P], f32, kind="ExternalInput")
    wbi_in = nc.dram_tensor("wbi", [P, 3, P], f32, kind="ExternalInput")
    out_t = nc.dram_tensor("out", [S, P], f32, kind="ExternalOutput")

    qn = [0]
    def next_q():
        q = qn[0]; qn[0] = (qn[0] + 1) % NQ
        return q

    i16 = lambda off: off // 16

    with tile.TileContext(nc) as tc:
        with tc.tile_pool(name="const", bufs=1) as cpool, \
             tc.tile_pool(name="v", bufs=2) as vpool, \
             tc.tile_pool(name="accA", bufs=1) as aApool, \
             tc.tile_pool(name="accB", bufs=1) as aBpool, \
             tc.tile_pool(name="accy", bufs=1) as ypool, \
             tc.tile_pool(name="outp", bufs=1) as opool, \
             tc.tile_pool(name="tmp", bufs=1) as tpool, \
             tc.tile_pool(name="g", bufs=8) as gpool, \
             tc.tile_pool(name="fin", bufs=2) as fpool, \
             tc.tile_pool(name="ps", bufs=4, space="PSUM") as pspool, \
             tc.tile_pool(name="dram", bufs=2, space="DRAM") as dpool, \
             tc.tile_pool(name="dramB", bufs=1, space="DRAM") as dBpool:

            idx_sb = cpool.tile([128, TOT // 16], mybir.dt.int16)
            nc.sync.dma_start(out=idx_sb[:], in_=idx_in[:])
            sc_sb = cpool.tile([P, 3, NB], f32)
            nc.sync.dma_start(out=sc_sb[:], in_=sc_in[:])
            tn_sb = cpool.tile([P, STEP, P], f32)
            nc.sync.dma_start(out=tn_sb[:], in_=tn_in[:])
            wbi_sb = cpool.tile([P, 3, P], f32)
            nc.sync.dma_start(out=wbi_sb[:], in_=wbi_in[:])

            def nscale(i):
                return sc_sb[:][:, i, :, None].to_broadcast([P, NB, P])

            def tnbc(s, nblk):
                return tn_sb[:][:, s, :][:, None, :].to_broadcast([P, nblk, P])

            def allgather(v_tile):
                # v_tile is [P, NB, P] fp16; shard DRAM layout is the flat
                # contiguous image of it ("(p b) f" rows).
                ag_in = dpool.tile([S, P], f16, tag="agin")
                nc.sync.dma_start(
                    out=ag_in[:].rearrange("(p b) f -> p b f", p=P),
                    in_=v_tile[:])
                ag_out = dpool.tile([Npad, P], f16, tag="agout",
                                    addr_space="Shared")
                if mock_collectives:
                    nc.sync.dma_start(out=ag_out[:][0:S, :], in_=ag_in[:])
                else:
                    nc.gpsimd.collective_compute(
                        "AllGather", mybir.AluOpType.bypass,
                        replica_groups=[list(range(NCORES))],
                        ins=[ag_in.opt()], outs=[ag_out.opt()])
                return ag_out

            gz_box = [None]

            def run_pass(plan, src, base, acct):
                for (t0, t1, pieces) in plan:
                    L = t1 - t0
                    g = (gz_box[0] if skip_gathers else
                         gpool.tile([P, CHUNK // P, P], f16, tag="g"))
                    if not skip_gathers:
                        nc.gpsimd.dma_gather(
                            out_ap=g[:, :L // P, :], in_ap=src,
                            idxs_ap=idx_sb[:, i16(base + t0):i16(base + t1)],
                            num_idxs=L, num_idxs_reg=L, elem_size=P,
                            queue_num=next_q())
                    if skip_adds:
                        continue
                    for (k, g_off, a_off, ln) in pieces:
                        gs = g[:, g_off // P:(g_off + ln) // P, :]
                        asl = acct[:, a_off // P:(a_off + ln) // P, :]
                        if k == 0:
                            nc.vector.tensor_copy(out=asl, in_=gs)
                        else:
                            nc.vector.tensor_add(out=asl, in0=asl, in1=gs)

            PA0 = int(meta["PKA"][0]); PB0 = int(meta["PKB"][0])

            for _rep in range(reps):
                if skip_gathers and gz_box[0] is None:
                    gz = cpool.tile([P, CHUNK // P, P], f16, tag="gz")
                    nc.vector.memset(gz[:], 0.0)
                    gz_box[0] = gz
                xt = tpool.tile([P, NB, P], f16, tag="xt")
                nc.sync.dma_start(out=xt[:],
                                  in_=x_in[:].rearrange("(p b) f -> p b f", p=P))
                v_sb = vpool.tile([P, NB, P], f16, tag="v")
                nc.vector.tensor_mul(out=v_sb[:], in0=xt[:], in1=nscale(0))
                acc_y = ypool.tile([P, NB, P], f32)
                nc.vector.tensor_mul(out=acc_y[:], in0=v_sb[:], in1=tnbc(0, NB))

                vfull = allgather(v_sb)

                for s in range(1, STEP):
                    accB = aBpool.tile([P, NB, P], f16, tag="accB")
                    if skip_adds:
                        nc.vector.memset(accB[:], 0.0)
                    run_pass(planB, vfull[HALF:Npad, :], TOTA, accB)
                    if PB0 < S:
                        nc.vector.memset(accB[:, PB0 // P:, :], 0.0)
                    accB_dram = dBpool.tile([S, P], f16, tag="accBd")
                    nc.sync.dma_start(
                        out=accB_dram[:].rearrange("(p b) f -> p b f", p=P),
                        in_=accB[:])
                    accA = aApool.tile([P, NB, P], f16, tag="accA")
                    if skip_adds:
                        nc.vector.memset(accA[:], 0.0)
                    run_pass(planA, vfull[0:HALF, :], 0, accA)
                    if PA0 < S:
                        nc.vector.memset(accA[:, PA0 // P:, :], 0.0)
                    mbase = TOTA + TOTB
                    t0 = 0
                    while t0 < S:
                        t1 = min(t0 + CHUNK, S)
                        L = t1 - t0
                        g = (gz_box[0] if skip_gathers else
                             gpool.tile([P, CHUNK // P, P], f16, tag="g"))
                        if not skip_gathers:
                            nc.gpsimd.dma_gather(
                                out_ap=g[:, :L // P, :], in_ap=accB_dram[:],
                                idxs_ap=idx_sb[:, i16(mbase + t0):i16(mbase + t1)],
                                num_idxs=L, num_idxs_reg=L, elem_size=P,
                                queue_num=next_q())
                        if not skip_adds:
                            nc.vector.tensor_add(
                                out=accA[:, t0 // P:t1 // P, :],
                                in0=accA[:, t0 // P:t1 // P, :],
                                in1=g[:, :L // P, :])
                        t0 = t1
                    nc.vector.tensor_add(out=accA[:], in0=accA[:], in1=v_sb[:])
                    v_next = vpool.tile([P, NB, P], f16, tag="v")
                    nc.vector.tensor_mul(out=v_next[:], in0=accA[:],
                                         in1=nscale(1))
                    if s < STEP - 1:
                        vfull = allgather(v_next)
                    BB = 8
                    for b0 in [] if skip_adds else range(0, NB, BB):
                        b1 = min(b0 + BB, NB)
                        tt = tpool.tile([P, BB, P], f32, tag="tt")
                        nc.vector.tensor_mul(out=tt[:, :b1 - b0, :],
                                             in0=v_next[:, b0:b1, :],
                                             in1=tnbc(s, b1 - b0))
                        nc.vector.tensor_add(out=acc_y[:, b0:b1, :],
                                             in0=acc_y[:, b0:b1, :],
                                             in1=tt[:, :b1 - b0, :])
                    v_sb = v_next

                out_sb = opool.tile([P, NB, P], f32)
                for b in range(NB):
                    ysc = fpool.tile([P, P], f32, tag="ysc")
                    nc.vector.tensor_mul(
                        out=ysc[:], in0=acc_y[:, b, :],
                        in1=sc_sb[:][:, 2, b, None].to_broadcast([P, P]))
                    yt_ps = pspool.tile([P, P], f32, tag="yt")
                    nc.tensor.transpose(out=yt_ps[:], in_=ysc[:],
                                        identity=wbi_sb[:][:, 2, :])
                    yt = fpool.tile([P, P], f32, tag="ytsb")
                    nc.vector.tensor_copy(out=yt[:], in_=yt_ps[:])
                    o_ps = pspool.tile([P, P], f32, tag="ops")
                    nc.tensor.matmul(out=o_ps[:], lhsT=yt[:],
                                     rhs=wbi_sb[:][:, 0, :],
                                     start=True, stop=True)
                    o_sb = out_sb[:, b, :]
                    nc.vector.tensor_add(out=o_sb, in0=o_ps[:],
                                         in1=wbi_sb[:][:, 1, :])
                    nc.vector.tensor_scalar_max(out=o_sb, in0=o_sb,
                                                scalar1=0.0)
                nc.sync.dma_start(
                    out=out_t[:].rearrange("(p b) f -> p b f", p=P),
                    in_=out_sb[:])

    nc.compile()
    return nc


# ---------------------------------------------------------------------------
# PJRT SPMD runner (mirrors concourse.bass2jax.run_bass_via_pjrt, persistent)
# ---------------------------------------------------------------------------

class _SpmdRunner:
    def __init__(self, nc, n_cores):
        import jax
        from jax.sharding import Mesh, PartitionSpec
        from jax.experimental.shard_map import shard_map
        from concourse import mybir
        from concourse.bass2jax import (_bass_exec_p, partition_id_tensor,
                                        install_neuronx_cc_hook)
        install_neuronx_cc_hook()
        self.jax = jax
        self.n_cores = n_cores
        partition_name = (nc.partition_id_tensor.name
                          if nc.partition_id_tensor else None)
        in_names, out_names, out_avals, zero_outs = [], [], [], []
        for alloc in nc.m.functions[0].allocations:
            if not isinstance(alloc, mybir.MemoryLocationSet):
                continue
            name = alloc.memorylocations[0].name
            if alloc.kind == "ExternalInput":
                if name != partition_name:
                    in_names.append(name)
            elif alloc.kind == "ExternalOutput":
                shape = tuple(alloc.tensor_shape)
                dtype = mybir.dt.np(alloc.dtype)
                out_names.append(name)
                out_avals.append(jax.core.ShapedArray(shape, dtype))
                zero_outs.append(np.zeros(shape, dtype))
        self.in_names, self.out_names = in_names, out_names
        self.out_avals, self.zero_outs = out_avals, zero_outs
        n_params, n_outs = len(in_names), len(out_names)
        self.n_params = n_params
        all_in = list(in_names) + list(out_names)
        if partition_name is not None:
            all_in.append(partition_name)

        def _body(*args):
            operands = list(args)
            if partition_name is not None:
                operands.append(partition_id_tensor())
            outs = _bass_exec_p.bind(
                *operands, out_avals=tuple(out_avals), in_names=tuple(all_in),
                out_names=tuple(out_names), lowering_input_output_aliases=(),
                sim_require_finite=True, sim_require_nnan=True, nc=nc)
            return tuple(outs)

        devices = jax.devices()[:n_cores]
        assert len(devices) == n_cores, \
            f"need {n_cores} neuron cores, have {len(jax.devices())}"
        mesh = Mesh(np.asarray(devices), ("core",))
        in_specs = (PartitionSpec("core"),) * (n_params + n_outs)
        out_specs = (PartitionSpec("core"),) * n_outs
        self.fn = jax.jit(
            shard_map(_body, mesh=mesh, in_specs=in_specs,
                      out_specs=out_specs, check_rep=False),
            keep_unused=True)

    def stage(self, in_maps):
        """device_put all inputs once; returns staged args for run_staged."""
        jax = self.jax
        per_core = [[np.asarray(m[n]) for n in self.in_names] for m in in_maps]
        concat_in = [np.concatenate([per_core[c][i] for c in range(self.n_cores)],
                                    axis=0) for i in range(self.n_params)]
        concat_zeros = [np.zeros((self.n_cores * z.shape[0], *z.shape[1:]), z.dtype)
                        for z in self.zero_outs]
        return [jax.device_put(a) for a in concat_in + concat_zeros]

    def run_staged(self, staged):
        outs = self.fn(*staged)
        self.jax.block_until_ready(outs)
        return outs

    def run(self, in_maps):
        jax = self.jax
        key = hash(tuple(hash(np.asarray(m[n]).tobytes())
                         for m in in_maps for n in self.in_names))
        staged = getattr(self, "_staged_cache", None)
        if staged is None or staged[0] != key:
            staged = (key, self.stage(in_maps))
            self._staged_cache = staged
        outs = self.run_staged(staged[1])
        res = []
        for c in range(self.n_cores):
            m = {}
            for i, name in enumerate(self.out_names):
                a = np.asarray(outs[i]).reshape(self.n_cores,
                                                *self.out_avals[i].shape)
                m[name] = a[c]
            res.append(m)
        return res


# ---------------------------------------------------------------------------
# public entry point
# ---------------------------------------------------------------------------

_CACHE = {}


def _get_compiled(edge_index, N, reps=1):
    key = (hash(np.asarray(edge_index).tobytes()), N, reps)
    if key not in _CACHE:
        meta = _preprocess(edge_index, N)
        nc = _build_program(meta, reps=reps)
        runner = _SpmdRunner(nc, NCORES)
        _CACHE[key] = (meta, runner)
    return _CACHE[key]


def _make_inputs(meta, x, t, init_weight, root_weight, bias):
    S = meta["S"]; Npad = meta["Npad"]; N = meta["N"]; NB = meta["NB"]
    tt = np.asarray(t, dtype=np.float32)
    e = np.exp(tt - tt.max(axis=0, keepdims=True))
    tnorm = (e / e.sum(axis=0, keepdims=True)).astype(np.float32)
    W = (np.asarray(init_weight[0]) +
         np.asarray(root_weight[0, 0])).astype(np.float32)
    bias_vec = np.asarray(bias[0, 0, 0], dtype=np.float32)

    tnorm_rep = np.ascontiguousarray(
        np.broadcast_to(tnorm[None], (P, STEP, P)), dtype=np.float32)
    wbi = np.ascontiguousarray(
        np.stack([W, np.broadcast_to(bias_vec[None, :], (P, P)),
                  np.eye(P, dtype=np.float32)], axis=1), dtype=np.float32)

    x_pad = np.zeros((Npad, P), dtype=np.float32)
    x_pad[:N] = np.asarray(x, dtype=np.float32)
    x_stor = x_pad[meta["node_of_stor"]]

    in_maps = []
    for c in range(NCORES):
        flat = np.concatenate([meta["tabsA"][c], meta["tabsB"][c],
                               meta["merge_idx"][c]])
        scales = np.ascontiguousarray(
            np.stack([meta["dinv_nm"][c], meta["dinv2_nm"][c],
                      meta["dinvinv_nm"][c]], axis=1), dtype=np.float32)
        # shard DRAM layout: node-major [P, NB, 128] flattened; storage slot
        # s lands at row (s%P)*NB + s//P.
        xs = x_stor[c * S:(c + 1) * S].reshape(NB, P, P).transpose(1, 0, 2)
        in_maps.append({
            "x": np.ascontiguousarray(
                xs.reshape(S, P)).astype(np.float16),
            "idx": _wrap_idx(flat),
            "scales": scales,
            "tnorm": tnorm_rep,
            "wbi": wbi,
        })
    return in_maps


def kernel(x, edge_index, t, init_weight, root_weight, bias):
    x = np.asarray(x)
    N = x.shape[0]
    meta, runner = _get_compiled(edge_index, N)
    in_maps = _make_inputs(meta, x, t, init_weight, root_weight, bias)
    res = runner.run(in_maps)
    S = meta["S"]; Npad = meta["Npad"]; NB = meta["NB"]
    full = np.empty((Npad, P), dtype=np.float32)
    for c in range(NCORES):
        # invert the node-major layout: row (p, b) -> storage slot b*P + p.
        o = res[c]["out"].reshape(P, NB, P).transpose(1, 0, 2).reshape(S, P)
        full[c * S:(c + 1) * S] = o
    out = np.empty((Npad, P), dtype=np.float32)
    out[meta["node_of_stor"]] = full
    return out[:N]


# revision 21
# speedup vs baseline: 1.4144x; 1.4144x over previous
"""ARMAPlusConv (10-step symmetric-normalized diffusion + fused linear head)
as a Trainium2 Bass/Tile kernel running SPMD on 8 NeuronCores.

Self-contained: kernel(**inputs) takes the full unsharded inputs
(x [N,128] f32, edge_index [2,E] i64, t [10,128] f32, init_weight [1,128,128],
root_weight [1,1,128,128], bias [1,1,1,128]) and returns the full
[N,128] f32 output.

Math (K=1; mean over K is the identity):
  deg[d] = 1 + indeg(d);  dinv = deg**-0.5
  v_0 = dinv * x
  v_s[d] = dinv2[d] * (sum_{src->d} v_{s-1}[src] + v_{s-1}[d]),  s=1..9
  y = (1/dinv) * sum_s tnorm[s] * v_s     (tnorm = softmax(t, axis=0))
  out = relu(y @ (Wi + Wr) + bias)

Distribution: nodes are dest-sharded over the 8 cores (S rows each, padded
with fake zero rows). v is carried in fp16 (tolerance 2e-2 allows it): this
halves the gather HBM traffic, the AllGather bytes, and doubles the DVE add
throughput. Each step, every core gathers its in-edge source rows from a
full replicated fp16 copy of v in its DRAM (refreshed by an AllGather of
the updated shards), segment-sums them with DVE adds, applies the self loop
and scales, and accumulates the softmax-weighted result in f32.

The sparse gather uses GPSIMD dma_gather (int16 indices, CHUNK per call,
4 SWDGE queues). Sources are split into two windows (cores 0-3 / cores 4-7)
so row indices fit int16. Each window uses degree-sorted prefix slots: slot
k holds the k-th in-edge source of every dest that has one; dests are
sorted by window-degree so slot k's dest list is a prefix and the gather
output (stream position q -> partition q%128, block q//128) lines up with
the accumulator exactly. Window A's dest order is the storage order; window
B accumulates separately (accB) in its own order and is merged back with
one S-row permute gather per step.

All bulk DRAM buffers (x, v shards, accB, out) use the node-major on-chip
layout [P, NB, 128] flattened to rows "(p b) f" so every bulk DMA is fully
contiguous; the gather index tables are built against that row numbering.
"""
import sys
sys.path.insert(0, "/opt/trn_rl_repo")
import numpy as np

P = 128
NCORES = 8
STEP = 10
CHUNK = 1024
NQ = 4
SCRATCH = 16384


# ---------------------------------------------------------------------------
# host preprocessing (graph structure only)
# ---------------------------------------------------------------------------

def _preprocess(edge_index, N):
    S = int(np.ceil(N / (NCORES * P))) * P
    Npad = NCORES * S
    HALF = (NCORES // 2) * S
    NB = S // P

    row = np.asarray(edge_index[0], dtype=np.int64)
    col = np.asarray(edge_index[1], dtype=np.int64)

    indeg = np.bincount(col, minlength=N)
    dinv_real = ((indeg + 1.0) ** -0.5).astype(np.float32)

    order = np.argsort(-indeg, kind="stable")
    rank_node = np.concatenate([order, np.arange(N, Npad)])
    members = [rank_node[np.arange(Npad) % NCORES == c] for c in range(NCORES)]
    node2core = np.empty(Npad, dtype=np.int64)
    for c in range(NCORES):
        node2core[members[c]] = c

    isA = node2core[row] < (NCORES // 2)
    degA = np.bincount(col[isA], minlength=Npad)
    degB = np.bincount(col[~isA], minlength=Npad)
    is_fake = np.zeros(Npad, dtype=bool)
    is_fake[N:] = True

    # storage slot s (0..S-1) of a node within its core; A-order == storage.
    stor_of_node = np.empty(Npad, dtype=np.int64)
    node_of_stor = np.empty(Npad, dtype=np.int64)
    piB_of_node = np.empty(Npad, dtype=np.int64)
    for c in range(NCORES):
        m = members[c]
        mA = m[np.lexsort((m, is_fake[m], -degA[m]))]
        stor_of_node[mA] = c * S + np.arange(S)
        node_of_stor[c * S + np.arange(S)] = mA
        mB = m[np.lexsort((m, is_fake[m], -degB[m]))]
        piB_of_node[mB] = np.arange(S)

    # DRAM row index of a storage slot: shard layout [P, NB, 128] flattened,
    # slot s -> partition s%P, block s//P -> row (s%P)*NB + s//P.
    def dram_row(slot):
        return (slot % P) * NB + slot // P

    st_src = stor_of_node[row]
    st_dst = stor_of_node[col]

    def build_window(mask, local_pos, deg_w, idx_base, zpad_stor):
        d_sel = st_dst[mask]
        s_sel = st_src[mask]
        o = np.argsort(d_sel, kind="stable")
        e_dst = d_sel[o]
        e_src = s_sel[o]
        starts = np.searchsorted(e_dst, np.arange(Npad))
        kk = np.arange(len(e_dst)) - starts[e_dst]
        Kmax = int(kk.max()) + 1 if len(kk) else 0
        pref = np.zeros((NCORES, Kmax), dtype=np.int64)
        for c in range(NCORES):
            dw = deg_w[members[c]]
            cnt = np.bincount(np.minimum(dw, Kmax), minlength=Kmax + 1)
            tail = np.cumsum(cnt[::-1])[::-1]
            pref[c] = tail[1:Kmax + 1]
        PK = ((pref.max(axis=0) + P - 1) // P) * P
        off = np.zeros(Kmax + 1, dtype=np.int64)
        off[1:] = np.cumsum(PK)
        # gather row index within the window: shard-local node-major rows.
        def win_row(stor):
            sl = stor - idx_base  # position within the window (0..HALF)
            core_off = (sl // S) * S
            return core_off + dram_row(sl % S)
        fill = win_row(np.int64(zpad_stor))
        tabs = np.full((NCORES, int(off[-1])), fill, dtype=np.int64)
        tabs[e_dst // S, off[kk] + local_pos[node_of_stor[e_dst]]] = \
            win_row(e_src)
        assert tabs.min() >= 0 and tabs.max() < HALF
        return tabs.astype(np.int16), PK

    posA = np.empty(Npad, dtype=np.int64)
    for c in range(NCORES):
        posA[node_of_stor[c * S:(c + 1) * S]] = np.arange(S)

    tabsA, PKA = build_window(isA, posA, degA, 0, HALF - 1)
    tabsB, PKB = build_window(~isA, piB_of_node, degB, HALF, Npad - 1)
    assert is_fake[node_of_stor[HALF - 1]] and is_fake[node_of_stor[Npad - 1]]

    # merge: storage slot s of core c holds node n with B-position piB(n);
    # accB_dram rows are node-major in B-position.
    merge_idx = np.empty((NCORES, S), dtype=np.int16)
    for c in range(NCORES):
        merge_idx[c] = dram_row(
            piB_of_node[node_of_stor[c * S:(c + 1) * S]]).astype(np.int16)

    nid = node_of_stor
    real = nid < N
    dinv_st = np.zeros(Npad, dtype=np.float32)
    dinv_st[real] = dinv_real[nid[real]]
    dinvinv_st = np.zeros(Npad, dtype=np.float32)
    dinvinv_st[real] = 1.0 / dinv_real[nid[real]]

    def node_major(a, c):
        return a[c * S:(c + 1) * S].reshape(NB, P).T.copy()

    return dict(
        N=N, S=S, Npad=Npad, HALF=HALF, NB=NB,
        node_of_stor=node_of_stor,
        tabsA=tabsA, PKA=PKA, tabsB=tabsB, PKB=PKB, merge_idx=merge_idx,
        dinv_nm=[node_major(dinv_st, c) for c in range(NCORES)],
        dinv2_nm=[node_major(dinv_st * dinv_st, c) for c in range(NCORES)],
        dinvinv_nm=[node_major(dinvinv_st, c) for c in range(NCORES)],
    )


def _chunk_plan(PK):
    offs = np.zeros(len(PK) + 1, dtype=np.int64)
    offs[1:] = np.cumsum(PK)
    TOT = int(offs[-1])
    plan = []
    t0 = 0
    while t0 < TOT:
        t1 = min(t0 + CHUNK, TOT)
        pieces = []
        for k, pk in enumerate(PK):
            a = max(int(offs[k]), t0)
            b = min(int(offs[k]) + int(pk), t1)
            if a < b:
                pieces.append((k, a - t0, a - int(offs[k]), b - a))
        plan.append((t0, t1, pieces))
        t0 = t1
    return plan, TOT


def _wrap_idx(flat):
    L = len(flat)
    return np.tile(flat.reshape(L // 16, 16).T, (8, 1)).copy()


# ---------------------------------------------------------------------------
# device program
# ---------------------------------------------------------------------------

def _build_program(meta, reps=1, mock_collectives=False,
                   skip_adds=False, skip_gathers=False, prep_trigger=False,
                   single_packet=True):
    from concourse import mybir, tile, bacc

    S = meta["S"]; HALF = meta["HALF"]; Npad = meta["Npad"]
    NB = S // P
    f32 = mybir.dt.float32
    f16 = mybir.dt.float16

    planA, TOTA = _chunk_plan(meta["PKA"])
    planB, TOTB = _chunk_plan(meta["PKB"])
    TOT = TOTA + TOTB + S

    nc = bacc.Bacc("TRN2", target_bir_lowering=False, debug=False,
                   num_devices=NCORES, num_swdge_queues=NQ,
                   dynamic_dma_scratch_size=SCRATCH)

    x_in = nc.dram_tensor("x", [S, P], f16, kind="ExternalInput")
    idx_in = nc.dram_tensor("idx", [128, TOT // 16], mybir.dt.int16,
                            kind="ExternalInput")
    sc_in = nc.dram_tensor("scales", [P, 3, NB], f32, kind="ExternalInput")
    tn_in = nc.dram_tensor("tnorm", [P, STEP, P], f32, kind="ExternalInput")
    wbi_in = nc.dram_tensor("wbi", [P, 3, P], f32, kind="ExternalInput")
    out_t = nc.dram_tensor("out", [S, P], f32, kind="ExternalOutput")

    qn = [0]
    def next_q():
        q = qn[0]; qn[0] = (qn[0] + 1) % NQ
        return q

    i16 = lambda off: off // 16

    with tile.TileContext(nc) as tc:
        with tc.tile_pool(name="const", bufs=1) as cpool, \
             tc.tile_pool(name="v", bufs=2) as vpool, \
             tc.tile_pool(name="accA", bufs=1) as aApool, \
             tc.tile_pool(name="accB", bufs=1) as aBpool, \
             tc.tile_pool(name="accy", bufs=1) as ypool, \
             tc.tile_pool(name="outp", bufs=1) as opool, \
             tc.tile_pool(name="tmp", bufs=1) as tpool, \
             tc.tile_pool(name="g", bufs=4) as gpool, \
             tc.tile_pool(name="fin", bufs=2) as fpool, \
             tc.tile_pool(name="ps", bufs=4, space="PSUM") as pspool, \
             tc.tile_pool(name="dram", bufs=2, space="DRAM") as dpool, \
             tc.tile_pool(name="dramB", bufs=1, space="DRAM") as dBpool:

            idx_sb = cpool.tile([128, TOT // 16], mybir.dt.int16)
            nc.sync.dma_start(out=idx_sb[:], in_=idx_in[:])
            sc_sb = cpool.tile([P, 3, NB], f32)
            nc.sync.dma_start(out=sc_sb[:], in_=sc_in[:])
            tn_sb = cpool.tile([P, STEP, P], f32)
            nc.sync.dma_start(out=tn_sb[:], in_=tn_in[:])
            wbi_sb = cpool.tile([P, 3, P], f32)
            nc.sync.dma_start(out=wbi_sb[:], in_=wbi_in[:])

            def nscale(i):
                return sc_sb[:][:, i, :, None].to_broadcast([P, NB, P])

            def tnbc(s, nblk):
                return tn_sb[:][:, s, :][:, None, :].to_broadcast([P, nblk, P])

            def allgather(v_tile):
                # v_tile is [P, NB, P] fp16; shard DRAM layout is the flat
                # contiguous image of it ("(p b) f" rows).
                ag_in = dpool.tile([S, P], f16, tag="agin")
                nc.sync.dma_start(
                    out=ag_in[:].rearrange("(p b) f -> p b f", p=P),
                    in_=v_tile[:])
                ag_out = dpool.tile([Npad, P], f16, tag="agout",
                                    addr_space="Shared")
                if mock_collectives:
                    nc.sync.dma_start(out=ag_out[:][0:S, :], in_=ag_in[:])
                else:
                    nc.gpsimd.collective_compute(
                        "AllGather", mybir.AluOpType.bypass,
                        replica_groups=[list(range(NCORES))],
                        ins=[ag_in.opt()], outs=[ag_out.opt()])
                return ag_out

            gz_box = [None]
            dma_sems = ([nc.alloc_semaphore(f"gsem{q}") for q in range(NQ)]
                        if prep_trigger else None)

            def do_gather(out_ap, in_ap, idxs_ap, L):
                q = next_q()
                if prep_trigger:
                    nc.gpsimd.dma_gather(
                        out_ap=out_ap, in_ap=in_ap, idxs_ap=idxs_ap,
                        num_idxs=L, num_idxs_reg=L, elem_size=P,
                        queue_num=q, prepare_only=True, sem=dma_sems[q],
                        single_packet=single_packet)
                    nc.gpsimd.trigger_dma(count=None, queue_num=q)
                else:
                    nc.gpsimd.dma_gather(
                        out_ap=out_ap, in_ap=in_ap, idxs_ap=idxs_ap,
                        num_idxs=L, num_idxs_reg=L, elem_size=P,
                        queue_num=q, single_packet=single_packet)

            def run_pass(plan, src, base, acct):
                for (t0, t1, pieces) in plan:
                    L = t1 - t0
                    g = (gz_box[0] if skip_gathers else
                         gpool.tile([P, CHUNK // P, P], f16, tag="g"))
                    if not skip_gathers:
                        do_gather(g[:, :L // P, :], src,
                                  idx_sb[:, i16(base + t0):i16(base + t1)], L)
                    if skip_adds:
                        continue
                    for (k, g_off, a_off, ln) in pieces:
                        gs = g[:, g_off // P:(g_off + ln) // P, :]
                        asl = acct[:, a_off // P:(a_off + ln) // P, :]
                        if k == 0:
                            nc.vector.tensor_copy(out=asl, in_=gs)
                        else:
                            nc.vector.tensor_add(out=asl, in0=asl, in1=gs)

            PA0 = int(meta["PKA"][0]); PB0 = int(meta["PKB"][0])

            for _rep in range(reps):
                if skip_gathers and gz_box[0] is None:
                    gz = cpool.tile([P, CHUNK // P, P], f16, tag="gz")
                    nc.vector.memset(gz[:], 0.0)
                    gz_box[0] = gz
                xt = tpool.tile([P, NB, P], f16, tag="xt")
                nc.sync.dma_start(out=xt[:],
                                  in_=x_in[:].rearrange("(p b) f -> p b f", p=P))
                v_sb = vpool.tile([P, NB, P], f16, tag="v")
                nc.vector.tensor_mul(out=v_sb[:], in0=xt[:], in1=nscale(0))
                acc_y = ypool.tile([P, NB, P], f32)
                nc.vector.tensor_mul(out=acc_y[:], in0=v_sb[:], in1=tnbc(0, NB))

                vfull = allgather(v_sb)

                for s in range(1, STEP):
                    accB = aBpool.tile([P, NB, P], f16, tag="accB")
                    if skip_adds:
                        nc.vector.memset(accB[:], 0.0)
                    run_pass(planB, vfull[HALF:Npad, :], TOTA, accB)
                    if PB0 < S:
                        nc.vector.memset(accB[:, PB0 // P:, :], 0.0)
                    accB_dram = dBpool.tile([S, P], f16, tag="accBd")
                    nc.sync.dma_start(
                        out=accB_dram[:].rearrange("(p b) f -> p b f", p=P),
                        in_=accB[:])
                    accA = aApool.tile([P, NB, P], f16, tag="accA")
                    if skip_adds:
                        nc.vector.memset(accA[:], 0.0)
                    run_pass(planA, vfull[0:HALF, :], 0, accA)
                    if PA0 < S:
                        nc.vector.memset(accA[:, PA0 // P:, :], 0.0)
                    mbase = TOTA + TOTB
                    t0 = 0
                    while t0 < S:
                        t1 = min(t0 + CHUNK, S)
                        L = t1 - t0
                        g = (gz_box[0] if skip_gathers else
                             gpool.tile([P, CHUNK // P, P], f16, tag="g"))
                        if not skip_gathers:
                            do_gather(g[:, :L // P, :], accB_dram[:],
                                      idx_sb[:, i16(mbase + t0):i16(mbase + t1)],
                                      L)
                        if not skip_adds:
                            nc.vector.tensor_add(
                                out=accA[:, t0 // P:t1 // P, :],
                                in0=accA[:, t0 // P:t1 // P, :],
                                in1=g[:, :L // P, :])
                        t0 = t1
                    nc.vector.tensor_add(out=accA[:], in0=accA[:], in1=v_sb[:])
                    v_next = vpool.tile([P, NB, P], f16, tag="v")
                    nc.vector.tensor_mul(out=v_next[:], in0=accA[:],
                                         in1=nscale(1))
                    if s < STEP - 1:
                        vfull = allgather(v_next)
                    BB = 8
                    for b0 in [] if skip_adds else range(0, NB, BB):
                        b1 = min(b0 + BB, NB)
                        tt = tpool.tile([P, BB, P], f32, tag="tt")
                        nc.vector.tensor_mul(out=tt[:, :b1 - b0, :],
                                             in0=v_next[:, b0:b1, :],
                                             in1=tnbc(s, b1 - b0))
                        nc.vector.tensor_add(out=acc_y[:, b0:b1, :],
                                             in0=acc_y[:, b0:b1, :],
                                             in1=tt[:, :b1 - b0, :])
                    v_sb = v_next

                out_sb = opool.tile([P, NB, P], f32)
                for b in range(NB):
                    ysc = fpool.tile([P, P], f32, tag="ysc")
                    nc.vector.tensor_mul(
                        out=ysc[:], in0=acc_y[:, b, :],
                        in1=sc_sb[:][:, 2, b, None].to_broadcast([P, P]))
                    yt_ps = pspool.tile([P, P], f32, tag="yt")
                    nc.tensor.transpose(out=yt_ps[:], in_=ysc[:],
                                        identity=wbi_sb[:][:, 2, :])
                    yt = fpool.tile([P, P], f32, tag="ytsb")
                    nc.vector.tensor_copy(out=yt[:], in_=yt_ps[:])
                    o_ps = pspool.tile([P, P], f32, tag="ops")
                    nc.tensor.matmul(out=o_ps[:], lhsT=yt[:],
                                     rhs=wbi_sb[:][:, 0, :],
                                     start=True, stop=True)
                    o_sb = out_sb[:, b, :]
                    nc.vector.tensor_add(out=o_sb, in0=o_ps[:],
                                         in1=wbi_sb[:][:, 1, :])
                    nc.vector.tensor_scalar_max(out=o_sb, in0=o_sb,
                                                scalar1=0.0)
                nc.sync.dma_start(
                    out=out_t[:].rearrange("(p b) f -> p b f", p=P),
                    in_=out_sb[:])

    nc.compile()
    return nc


# ---------------------------------------------------------------------------
# PJRT SPMD runner (mirrors concourse.bass2jax.run_bass_via_pjrt, persistent)
# ---------------------------------------------------------------------------

class _SpmdRunner:
    def __init__(self, nc, n_cores):
        import jax
        from jax.sharding import Mesh, PartitionSpec
        from jax.experimental.shard_map import shard_map
        from concourse import mybir
        from concourse.bass2jax import (_bass_exec_p, partition_id_tensor,
                                        install_neuronx_cc_hook)
        install_neuronx_cc_hook()
        self.jax = jax
        self.n_cores = n_cores
        partition_name = (nc.partition_id_tensor.name
                          if nc.partition_id_tensor else None)
        in_names, out_names, out_avals, zero_outs = [], [], [], []
        for alloc in nc.m.functions[0].allocations:
            if not isinstance(alloc, mybir.MemoryLocationSet):
                continue
            name = alloc.memorylocations[0].name
            if alloc.kind == "ExternalInput":
                if name != partition_name:
                    in_names.append(name)
            elif alloc.kind == "ExternalOutput":
                shape = tuple(alloc.tensor_shape)
                dtype = mybir.dt.np(alloc.dtype)
                out_names.append(name)
                out_avals.append(jax.core.ShapedArray(shape, dtype))
                zero_outs.append(np.zeros(shape, dtype))
        self.in_names, self.out_names = in_names, out_names
        self.out_avals, self.zero_outs = out_avals, zero_outs
        n_params, n_outs = len(in_names), len(out_names)
        self.n_params = n_params
        all_in = list(in_names) + list(out_names)
        if partition_name is not None:
            all_in.append(partition_name)

        def _body(*args):
            operands = list(args)
            if partition_name is not None:
                operands.append(partition_id_tensor())
            outs = _bass_exec_p.bind(
                *operands, out_avals=tuple(out_avals), in_names=tuple(all_in),
                out_names=tuple(out_names), lowering_input_output_aliases=(),
                sim_require_finite=True, sim_require_nnan=True, nc=nc)
            return tuple(outs)

        devices = jax.devices()[:n_cores]
        assert len(devices) == n_cores, \
            f"need {n_cores} neuron cores, have {len(jax.devices())}"
        mesh = Mesh(np.asarray(devices), ("core",))
        in_specs = (PartitionSpec("core"),) * (n_params + n_outs)
        out_specs = (PartitionSpec("core"),) * n_outs
        self.fn = jax.jit(
            shard_map(_body, mesh=mesh, in_specs=in_specs,
                      out_specs=out_specs, check_rep=False),
            keep_unused=True)

    def stage(self, in_maps):
        """device_put all inputs once; returns staged args for run_staged."""
        jax = self.jax
        per_core = [[np.asarray(m[n]) for n in self.in_names] for m in in_maps]
        concat_in = [np.concatenate([per_core[c][i] for c in range(self.n_cores)],
                                    axis=0) for i in range(self.n_params)]
        concat_zeros = [np.zeros((self.n_cores * z.shape[0], *z.shape[1:]), z.dtype)
                        for z in self.zero_outs]
        return [jax.device_put(a) for a in concat_in + concat_zeros]

    def run_staged(self, staged):
        outs = self.fn(*staged)
        self.jax.block_until_ready(outs)
        return outs

    def run(self, in_maps):
        jax = self.jax
        key = hash(tuple(hash(np.asarray(m[n]).tobytes())
                         for m in in_maps for n in self.in_names))
        staged = getattr(self, "_staged_cache", None)
        if staged is None or staged[0] != key:
            staged = (key, self.stage(in_maps))
            self._staged_cache = staged
        outs = self.run_staged(staged[1])
        res = []
        for c in range(self.n_cores):
            m = {}
            for i, name in enumerate(self.out_names):
                a = np.asarray(outs[i]).reshape(self.n_cores,
                                                *self.out_avals[i].shape)
                m[name] = a[c]
            res.append(m)
        return res


# ---------------------------------------------------------------------------
# public entry point
# ---------------------------------------------------------------------------

_CACHE = {}


def _get_compiled(edge_index, N, reps=1):
    key = (hash(np.asarray(edge_index).tobytes()), N, reps)
    if key not in _CACHE:
        meta = _preprocess(edge_index, N)
        nc = _build_program(meta, reps=reps)
        runner = _SpmdRunner(nc, NCORES)
        _CACHE[key] = (meta, runner)
    return _CACHE[key]


def _make_inputs(meta, x, t, init_weight, root_weight, bias):
    S = meta["S"]; Npad = meta["Npad"]; N = meta["N"]; NB = meta["NB"]
    tt = np.asarray(t, dtype=np.float32)
    e = np.exp(tt - tt.max(axis=0, keepdims=True))
    tnorm = (e / e.sum(axis=0, keepdims=True)).astype(np.float32)
    W = (np.asarray(init_weight[0]) +
         np.asarray(root_weight[0, 0])).astype(np.float32)
    bias_vec = np.asarray(bias[0, 0, 0], dtype=np.float32)

    tnorm_rep = np.ascontiguousarray(
        np.broadcast_to(tnorm[None], (P, STEP, P)), dtype=np.float32)
    wbi = np.ascontiguousarray(
        np.stack([W, np.broadcast_to(bias_vec[None, :], (P, P)),
                  np.eye(P, dtype=np.float32)], axis=1), dtype=np.float32)

    x_pad = np.zeros((Npad, P), dtype=np.float32)
    x_pad[:N] = np.asarray(x, dtype=np.float32)
    x_stor = x_pad[meta["node_of_stor"]]

    in_maps = []
    for c in range(NCORES):
        flat = np.concatenate([meta["tabsA"][c], meta["tabsB"][c],
                               meta["merge_idx"][c]])
        scales = np.ascontiguousarray(
            np.stack([meta["dinv_nm"][c], meta["dinv2_nm"][c],
                      meta["dinvinv_nm"][c]], axis=1), dtype=np.float32)
        # shard DRAM layout: node-major [P, NB, 128] flattened; storage slot
        # s lands at row (s%P)*NB + s//P.
        xs = x_stor[c * S:(c + 1) * S].reshape(NB, P, P).transpose(1, 0, 2)
        in_maps.append({
            "x": np.ascontiguousarray(
                xs.reshape(S, P)).astype(np.float16),
            "idx": _wrap_idx(flat),
            "scales": scales,
            "tnorm": tnorm_rep,
            "wbi": wbi,
        })
    return in_maps


def kernel(x, edge_index, t, init_weight, root_weight, bias):
    x = np.asarray(x)
    N = x.shape[0]
    meta, runner = _get_compiled(edge_index, N)
    in_maps = _make_inputs(meta, x, t, init_weight, root_weight, bias)
    res = runner.run(in_maps)
    S = meta["S"]; Npad = meta["Npad"]; NB = meta["NB"]
    full = np.empty((Npad, P), dtype=np.float32)
    for c in range(NCORES):
        # invert the node-major layout: row (p, b) -> storage slot b*P + p.
        o = res[c]["out"].reshape(P, NB, P).transpose(1, 0, 2).reshape(S, P)
        full[c * S:(c + 1) * S] = o
    out = np.empty((Npad, P), dtype=np.float32)
    out[meta["node_of_stor"]] = full
    return out[:N]
